# revision 9
# baseline (speedup 1.0000x reference)
"""GCN (3-layer) Bass kernel for Trainium2, 8 NeuronCores.

Reference computation (B=8192, IN=64, HID=128, OUT=64):
    A = binarize(bone_adj); A[diag] = 1
    deg = A.sum(axis=0); dinv = rsqrt(deg)
    N = dinv[:,None] * A * dinv[None,:]
    x = features; for (W, b) in layers: x = relu(N.T @ (x @ W) + b)

Kernel strategy:
  - Column-shard A across 8 cores: core c owns target nodes t in
    [1024c, 1024(c+1)).  Host casts the binary A to bf16 (exact 0/1).
  - Each core keeps its whole 16MB bf16 shard resident in SBUF (loaded once).
  - deg[t] = sum_s A[s,t] computed on device via ones-vector matmul, fused
    with (pipelined under) the A load; deg slices are AllGathered so every
    core has the full dinv vector.
  - Per layer, with Z = dinv ⊙ Y_{prev} kept in bf16 [8192, F]:
      U_t = Z.T @ A_shard      (PE: Z s-block stationary, A moving)
      Y_pre[t, fo] = (U @ W)[t, fo] + sqrt(deg[t]) * b[fo]   (small f32 matmuls)
      AG'd next-Z = relu(dinv_t^2 * Y_pre)    (= dinv ⊙ relu(dinv ⊙ (UW) + b))
      final out   = relu(dinv_t   * Y_pre)
  - Activations are AllGathered between layers (256KB per rank).
"""

import numpy as np
import ml_dtypes

B, IN, HID, OUT = 8192, 64, 128, 64
NCORES = 8
TS = B // NCORES  # 1024 targets per core
P = 128
SB = B // P  # 64 source blocks
GRP = 8  # z group tiles: 8 groups of 8 s-blocks

_CACHE = {}


def _build():
    import concourse.bass as bass
    import concourse.mybir as mybir
    import concourse.tile as tile

    dt = mybir.dt
    AF = mybir.ActivationFunctionType

    nc = bass.Bass(num_devices=NCORES)

    # ---- I/O ----
    a_sh = nc.dram_tensor("a_sh", [B, TS], dt.bfloat16, kind="ExternalInput")
    x_in = nc.dram_tensor("x_in", [B, IN], dt.float32, kind="ExternalInput")
    w0_d = nc.dram_tensor("w0", [IN, HID], dt.float32, kind="ExternalInput")
    w1_d = nc.dram_tensor("w1", [HID, HID], dt.float32, kind="ExternalInput")
    w2_d = nc.dram_tensor("w2", [HID, OUT], dt.float32, kind="ExternalInput")
    b0_d = nc.dram_tensor("b0", [1, HID], dt.float32, kind="ExternalInput")
    b1_d = nc.dram_tensor("b1", [1, HID], dt.float32, kind="ExternalInput")
    b2_d = nc.dram_tensor("b2", [1, OUT], dt.float32, kind="ExternalInput")
    out_sh = nc.dram_tensor("out_sh", [TS, OUT], dt.float32, kind="ExternalOutput")

    rg = [list(range(NCORES))]

    with tile.TileContext(nc) as tc:
        with (
            tc.tile_pool(name="dram", bufs=1, space="DRAM") as dram,
            tc.tile_pool(name="apool", bufs=SB) as apool,
            tc.tile_pool(name="zpool", bufs=1) as zpool,
            tc.tile_pool(name="xpool", bufs=3) as xpool,
            tc.tile_pool(name="spool", bufs=1) as spool,
            tc.tile_pool(name="upool", bufs=2) as upool,
            tc.tile_pool(name="pdeg", bufs=1, space="PSUM") as pdeg,
            tc.tile_pool(name="pu", bufs=1, space="PSUM") as pu,
            tc.tile_pool(name="py", bufs=2, space="PSUM") as py,
        ):
            # ---- collective bounce buffers (DRAM) ----
            deg_bounce = dram.tile([1, TS], dt.float32, name="deg_bounce")
            deg_all = dram.tile(
                [NCORES, TS], dt.float32, addr_space="Shared", name="deg_all"
            )
            zb = [
                dram.tile([TS, HID], dt.bfloat16, name=f"zb{i}", tag=f"zb{i}")
                for i in range(2)
            ]
            zall = [
                dram.tile(
                    [B, HID], dt.bfloat16, addr_space="Shared",
                    name=f"zall{i}", tag=f"zall{i}",
                )
                for i in range(2)
            ]

            # ---- constants ----
            ones_bf = spool.tile([P, 1], dt.bfloat16, name="ones_bf")
            nc.vector.memset(ones_bf[:], 1.0)

            # ---- phase A: load A shard (SBUF-resident) + degree matmuls ----
            a_tiles = []
            a_view = a_sh[:].rearrange("(k p) t -> k p t", p=P)
            deg_ps0 = pdeg.tile([1, 512], dt.float32, name="deg_ps0", tag="deg0")
            deg_ps1 = pdeg.tile([1, 512], dt.float32, name="deg_ps1", tag="deg1")
            for k in range(SB):
                at = apool.tile([P, TS], dt.bfloat16, name=f"a{k}", tag="a")
                nc.sync.dma_start(at[:], a_view[k])
                a_tiles.append(at)
            for k in range(SB):
                nc.tensor.matmul(
                    deg_ps0[:], ones_bf[:], a_tiles[k][:, 0:512],
                    start=(k == 0), stop=(k == SB - 1),
                )
                nc.tensor.matmul(
                    deg_ps1[:], ones_bf[:], a_tiles[k][:, 512:TS],
                    start=(k == 0), stop=(k == SB - 1),
                )

            # prefetch x / W / b while A loads
            x_view = x_in[:].rearrange("(k p) f -> p k f", p=P)
            xg_tiles = []
            for g in range(GRP):
                xg = xpool.tile([P, GRP, IN], dt.float32, name=f"xg{g}", tag="xg")
                nc.sync.dma_start(xg[:], x_view[:, g * GRP:(g + 1) * GRP, :])
                xg_tiles.append(xg)
            w_sb = []
            for i, (wd, fi, fo) in enumerate(
                ((w0_d, IN, HID), (w1_d, HID, HID), (w2_d, HID, OUT))
            ):
                wt = spool.tile([fi, fo], dt.float32, name=f"w{i}sb", tag=f"w{i}")
                nc.sync.dma_start(wt[:], wd[:])
                w_sb.append(wt)
            b_sb = []
            for i, (bd, fo) in enumerate(((b0_d, HID), (b1_d, HID), (b2_d, OUT))):
                bt = spool.tile([1, fo], dt.float32, name=f"b{i}sb", tag=f"b{i}")
                nc.sync.dma_start(bt[:], bd[:])
                b_sb.append(bt)

            # ---- degree -> AllGather -> dinv ----
            deg_sb = spool.tile([1, TS], dt.float32, name="deg_sb")
            nc.vector.tensor_copy(deg_sb[:, 0:512], deg_ps0[:])
            nc.vector.tensor_copy(deg_sb[:, 512:TS], deg_ps1[:])
            nc.sync.dma_start(deg_bounce[:], deg_sb[:])
            nc.gpsimd.collective_compute(
                "AllGather",
                mybir.AluOpType.bypass,
                replica_groups=rg,
                ins=[deg_bounce[:].opt()],
                outs=[deg_all[:].opt()],
            )

            # sqrt(deg[t]) row for bias prefill (local slice)
            sqd_row = spool.tile([1, TS], dt.float32, name="sqd_row")
            nc.scalar.activation(sqd_row[:], deg_sb[:], AF.Sqrt)

            # dinv for all sources: [128, 64] with element (p, k) = dinv[k*128+p]
            dinv_s = spool.tile([P, SB], dt.float32, name="dinv_s")
            nc.sync.dma_start(
                dinv_s[:], deg_all[:].rearrange("c (q p) -> p (c q)", p=P)
            )
            nc.scalar.activation(dinv_s[:], dinv_s[:], AF.Sqrt)
            nc.vector.reciprocal(dinv_s[:], dinv_s[:])

            # dinv for own targets: [128, 8], col j covers t-block j
            dinv_t = spool.tile([P, GRP], dt.float32, name="dinv_t")
            nc.sync.dma_start(
                dinv_t[:], deg_bounce[:].rearrange("o (j p) -> p (o j)", p=P)
            )
            nc.scalar.activation(dinv_t[:], dinv_t[:], AF.Sqrt)
            nc.vector.reciprocal(dinv_t[:], dinv_t[:])
            dinv2_t = spool.tile([P, GRP], dt.float32, name="dinv2_t")
            nc.vector.tensor_mul(dinv2_t[:], dinv_t[:], dinv_t[:])

            # ---- z1 = dinv ⊙ x  (bf16, group tiles) ----
            z_tiles = []
            for g in range(GRP):
                zg = zpool.tile(
                    [P, GRP * IN], dt.bfloat16, name=f"z1g{g}", tag="zg", bufs=10
                )
                scale = (
                    dinv_s[:, g * GRP:(g + 1) * GRP]
                    .unsqueeze(2)
                    .broadcast_to([P, GRP, IN])
                )
                nc.vector.tensor_tensor(
                    zg[:].rearrange("p (k f) -> p k f", f=IN),
                    xg_tiles[g][:],
                    scale,
                    mybir.AluOpType.mult,
                )
                z_tiles.append(zg)

            # ---- layers ----
            def layer(li, f_in, f_out, z_tiles_in):
                u_ps0 = pu.tile([f_in, 512], dt.float32, name=f"u{li}ps0", tag="u0")
                u_ps1 = pu.tile([f_in, 512], dt.float32, name=f"u{li}ps1", tag="u1")
                for k in range(SB):
                    zt = z_tiles_in[k // GRP][:, (k % GRP) * f_in:(k % GRP + 1) * f_in]
                    nc.tensor.matmul(
                        u_ps0[:], zt, a_tiles[k][:, 0:512],
                        start=(k == 0), stop=(k == SB - 1),
                    )
                    nc.tensor.matmul(
                        u_ps1[:], zt, a_tiles[k][:, 512:TS],
                        start=(k == 0), stop=(k == SB - 1),
                    )
                u_sb = upool.tile([f_in, TS], dt.float32, name=f"u{li}sb", tag="u")
                nc.vector.tensor_copy(u_sb[:, 0:512], u_ps0[:])
                nc.vector.tensor_copy(u_sb[:, 512:TS], u_ps1[:])

                last = li == 2
                odt = dt.float32 if last else dt.bfloat16
                y_sb = zpool.tile(
                    [P, GRP * f_out], odt, name=f"y{li}sb",
                    tag="yout" if last else "zg",
                    bufs=1 if last else 10,
                )
                for j in range(GRP):
                    y_ps = py.tile([P, f_out], dt.float32, name=f"y{li}ps{j}", tag="y")
                    # PSUM prefill: sqrt(deg[t]) * b[fo]
                    nc.tensor.matmul(
                        y_ps[:],
                        sqd_row[0:1, j * P:(j + 1) * P],
                        b_sb[li][:],
                        start=True, stop=False,
                    )
                    nc.tensor.matmul(
                        y_ps[:],
                        u_sb[:, j * P:(j + 1) * P],
                        w_sb[li][:],
                        start=False, stop=True,
                    )
                    scale = (dinv_t if last else dinv2_t)[:, j:j + 1]
                    nc.scalar.activation(
                        y_sb[:, j * f_out:(j + 1) * f_out],
                        y_ps[:],
                        AF.Relu,
                        scale=scale,
                    )
                return y_sb

            for li, (f_in, f_out) in enumerate(((IN, HID), (HID, HID), (HID, OUT))):
                y_sb = layer(li, f_in, f_out, z_tiles)
                if li == 2:
                    nc.sync.dma_start(
                        out_sh[:].rearrange("(j p) f -> p j f", p=P),
                        y_sb[:].rearrange("p (j f) -> p j f", f=OUT),
                    )
                else:
                    nc.sync.dma_start(
                        zb[li][:].rearrange("(j p) f -> p j f", p=P),
                        y_sb[:].rearrange("p (j f) -> p j f", f=f_out),
                    )
                    nc.gpsimd.collective_compute(
                        "AllGather",
                        mybir.AluOpType.bypass,
                        replica_groups=rg,
                        ins=[zb[li][:].opt()],
                        outs=[zall[li][:].opt()],
                    )
                    zall_view = zall[li][:].rearrange(
                        "(g kk p) f -> g p kk f", g=GRP, p=P
                    )
                    z_tiles = []
                    for g in range(GRP):
                        zg = zpool.tile(
                            [P, GRP * f_out], dt.bfloat16, name=f"z{li}g{g}",
                            tag="zg", bufs=10,
                        )
                        nc.sync.dma_start(
                            zg[:].rearrange("p (k f) -> p k f", f=f_out),
                            zall_view[g],
                        )
                        z_tiles.append(zg)

    _split_sync_waits(nc)
    return nc


def _split_sync_waits(nc, limit=1):
    """This container's walrus rejects >1 sync wait per instruction
    ("Too many sync wait commands").  Hoist extra waits onto NoOps that
    immediately precede the instruction on the same engine — semantically
    identical since waits on one engine sequencer serialize anyway."""
    import concourse.mybir as mybir

    n_split = 0
    for f in nc.m.functions:
        for bb in f.blocks:
            out = []
            for ins in bb.instructions:
                si = ins.sync_info
                waits = list(si.on_wait) if si is not None and si.on_wait else []
                if len(waits) > limit:
                    extra, keep = waits[:-limit], waits[-limit:]
                    for i, w in enumerate(extra):
                        out.append(
                            mybir.InstNoOp(
                                name=f"{ins.name}-ws{i}",
                                engine=ins.engine,
                                bass_nofuse=True,
                                sync_info=mybir.SyncInfo(on_wait=[w], on_update=[]),
                            )
                        )
                    ins.sync_info = mybir.SyncInfo(
                        on_wait=keep, on_update=list(si.on_update)
                    )
                    n_split += 1
                out.append(ins)
            bb.instructions = out
    return n_split


def _prep_shards(bone_features, bone_adj, W0, b0, W1, b1, W2, b2):
    bf16 = ml_dtypes.bfloat16
    a_bin = (bone_adj != 0).astype(bf16)
    np.fill_diagonal(a_bin, bf16(1.0))
    x = np.ascontiguousarray(bone_features, dtype=np.float32)
    common = dict(
        x_in=x,
        w0=np.ascontiguousarray(W0, dtype=np.float32),
        w1=np.ascontiguousarray(W1, dtype=np.float32),
        w2=np.ascontiguousarray(W2, dtype=np.float32),
        b0=np.ascontiguousarray(b0, dtype=np.float32).reshape(1, HID),
        b1=np.ascontiguousarray(b1, dtype=np.float32).reshape(1, HID),
        b2=np.ascontiguousarray(b2, dtype=np.float32).reshape(1, OUT),
    )
    in_maps = []
    for c in range(NCORES):
        shard = np.ascontiguousarray(a_bin[:, c * TS:(c + 1) * TS])
        in_maps.append(dict(a_sh=shard, **common))
    return in_maps


def _ensure_ntff_hook():
    """The image's antenv lacks axon_hooks, so boot() skipped registering the
    NTFF profile hook.  Recreate the module and register the hook so
    run_bass_kernel_spmd(trace=True) can profile."""
    import sys
    import types

    if "antenv.axon_hooks" in sys.modules:
        return
    mod = types.ModuleType("antenv.axon_hooks")
    hook = [None]
    mod.set_axon_ntff_profile_hook = lambda h: hook.__setitem__(0, h)
    mod.get_axon_ntff_profile_hook = lambda: hook[0]
    sys.modules["antenv.axon_hooks"] = mod
    import antenv

    antenv.axon_hooks = mod
    try:
        from trn_agent_boot.trn_boot import _ntff_profile_via_ctypes

        mod.set_axon_ntff_profile_hook(
            _ntff_profile_via_ctypes("/opt/axon/libaxon_pjrt.so")
        )
    except Exception:
        pass


def run(trace=False, **inputs):
    from concourse.bass_utils import run_bass_kernel_spmd

    if trace:
        _ensure_ntff_hook()
    if "nc" not in _CACHE:
        _CACHE["nc"] = _build()
    nc = _CACHE["nc"]
    in_maps = _prep_shards(**inputs)
    res = run_bass_kernel_spmd(
        nc, in_maps, list(range(NCORES)), trace=trace,
        trace_cores=list(range(NCORES)) if trace else None,
    )
    out = np.concatenate(
        [np.asarray(res.results[c]["out_sh"]) for c in range(NCORES)], axis=0
    )
    return out, res


def kernel(**inputs):
    out, _ = run(trace=False, **inputs)
    return out


# revision 12
# speedup vs baseline: 1.0459x; 1.0459x over previous
"""GCN (3-layer) Bass kernel for Trainium2, 8 NeuronCores.

Reference computation (B=8192, IN=64, HID=128, OUT=64):
    A = binarize(bone_adj); A[diag] = 1
    deg = A.sum(axis=0); dinv = rsqrt(deg)
    N = dinv[:,None] * A * dinv[None,:]
    x = features; for (W, b) in layers: x = relu(N.T @ (x @ W) + b)

Kernel strategy:
  - Column-shard A across 8 cores: core c owns target nodes t in
    [1024c, 1024(c+1)).  Host casts the binary A to fp8e4m3 (exact 0/1).
  - Each core keeps its whole 8MB fp8 shard resident in SBUF (loaded once).
  - deg[t] = sum_s A[s,t] for the core's own targets is a column sum of its
    own shard — computed on device via ones-vector matmuls pipelined under
    the A load.  No degree AllGather is needed: every consumer of dinv[s]
    gets it pre-applied by the owner of s (see below).
  - Activations cross cores pre-scaled: the AllGathered tensor for layer l
    is Z = dinv ⊙ Y_{l-1} (bf16), where each core scales its own slice with
    its locally-computed dinv.  Layer l computes:
      U_t = Z.T @ A_shard        (PE: Z s-block stationary bf16, A moving fp8)
      Y_pre[t, fo] = (U @ W)[t, fo] + sqrt(deg[t]) * b[fo]  (small f32 matmuls;
        the sqrt(deg) prefactor cancels the later dinv_t scale on the bias)
      AG input     = relu(dinv_t^2 * Y_pre)   (= dinv ⊙ relu(dinv ⊙ (UW) + b))
      final output = relu(dinv_t   * Y_pre)
  - A short burst of dummy matmuls at kernel start warms the PE clock gate
    (HAM) so the degree pass runs at 2.4 GHz.
"""

import numpy as np
import ml_dtypes

B, IN, HID, OUT = 8192, 64, 128, 64
NCORES = 8
TS = B // NCORES  # 1024 targets per core
P = 128
SB = B // P  # 64 source blocks
NPAIR = SB // 2  # A held as 32 pair-tiles [128, 2, 1024]
GRP = 8  # z group tiles: 8 groups of 8 s-blocks

_CACHE = {}


def _build():
    import concourse.bass as bass
    import concourse.mybir as mybir
    import concourse.tile as tile

    dt = mybir.dt
    AF = mybir.ActivationFunctionType

    nc = bass.Bass(num_devices=NCORES)

    # ---- I/O ----
    a_sh = nc.dram_tensor("a_sh", [B, TS], dt.float8e4, kind="ExternalInput")
    x_sl = nc.dram_tensor("x_sl", [TS, IN], dt.float32, kind="ExternalInput")
    w0_d = nc.dram_tensor("w0", [IN, HID], dt.float32, kind="ExternalInput")
    w1_d = nc.dram_tensor("w1", [HID, HID], dt.float32, kind="ExternalInput")
    w2_d = nc.dram_tensor("w2", [HID, OUT], dt.float32, kind="ExternalInput")
    b0_d = nc.dram_tensor("b0", [1, HID], dt.float32, kind="ExternalInput")
    b1_d = nc.dram_tensor("b1", [1, HID], dt.float32, kind="ExternalInput")
    b2_d = nc.dram_tensor("b2", [1, OUT], dt.float32, kind="ExternalInput")
    out_sh = nc.dram_tensor("out_sh", [TS, OUT], dt.float32, kind="ExternalOutput")

    rg = [list(range(NCORES))]

    with tile.TileContext(nc) as tc:
        with (
            tc.tile_pool(name="dram", bufs=1, space="DRAM") as dram,
            tc.tile_pool(name="apool", bufs=NPAIR) as apool,
            tc.tile_pool(name="zpool", bufs=1) as zpool,
            tc.tile_pool(name="spool", bufs=1) as spool,
            tc.tile_pool(name="upool", bufs=2) as upool,
            tc.tile_pool(name="pdeg", bufs=1, space="PSUM") as pdeg,
            tc.tile_pool(name="pu", bufs=1, space="PSUM") as pu,
            tc.tile_pool(name="py", bufs=2, space="PSUM") as py,
        ):
            # ---- collective bounce buffers (DRAM) ----
            fdims = (IN, HID, HID)
            zb = [
                dram.tile([TS, fdims[i]], dt.bfloat16, name=f"zb{i}", tag=f"zb{i}")
                for i in range(3)
            ]
            zall = [
                dram.tile(
                    [B, fdims[i]], dt.bfloat16, addr_space="Shared",
                    name=f"zall{i}", tag=f"zall{i}",
                )
                for i in range(3)
            ]
            deg_bounce = dram.tile([1, TS], dt.float32, name="deg_bounce")

            # ---- constants + PE warmup (HAM: ~4us of matmuls -> 2.4 GHz) ----
            ones_bf = spool.tile([P, 1], dt.bfloat16, name="ones_bf")
            nc.vector.memset(ones_bf[:], 1.0)
            junk = spool.tile([P, 512], dt.bfloat16, name="junk")
            nc.vector.memset(junk[:], 0.0)
            import os as _os
            if not _os.environ.get("K_NO_WARMUP"):
                wu_ps = pdeg.tile([1, 512], dt.float32, name="wu_ps", tag="wu")
                for _ in range(20):
                    nc.tensor.matmul(
                        wu_ps[:], ones_bf[:], junk[:], start=True, stop=True
                    )

            # ---- phase A: load A shard (SBUF-resident, fp8) + degree pass ----
            a_tiles = []
            a_view = a_sh[:].rearrange("(k h p) t -> k p h t", h=2, p=P)
            deg_ps0 = pdeg.tile([1, 512], dt.float32, name="deg_ps0", tag="deg0")
            deg_ps1 = pdeg.tile([1, 512], dt.float32, name="deg_ps1", tag="deg1")
            for k in range(NPAIR):
                at = apool.tile([P, 2, TS], dt.float8e4, name=f"a{k}", tag="a")
                nc.sync.dma_start(at[:], a_view[k])
                a_tiles.append(at)
            for k in range(NPAIR):
                for h in range(2):
                    nc.tensor.matmul(
                        deg_ps0[:], ones_bf[:], a_tiles[k][:, h, 0:512],
                        start=(k == 0 and h == 0), stop=(k == NPAIR - 1 and h == 1),
                    )
                    nc.tensor.matmul(
                        deg_ps1[:], ones_bf[:], a_tiles[k][:, h, 512:TS],
                        start=(k == 0 and h == 0), stop=(k == NPAIR - 1 and h == 1),
                    )

            # prefetch x / W / b while A loads
            xj = spool.tile([P, GRP, IN], dt.float32, name="xj")
            nc.sync.dma_start(
                xj[:], x_sl[:].rearrange("(j p) f -> p j f", p=P)
            )
            w_sb = []
            for i, (wd, fi, fo) in enumerate(
                ((w0_d, IN, HID), (w1_d, HID, HID), (w2_d, HID, OUT))
            ):
                wt = spool.tile([fi, fo], dt.float32, name=f"w{i}sb", tag=f"w{i}")
                nc.sync.dma_start(wt[:], wd[:])
                w_sb.append(wt)
            b_sb = []
            for i, (bd, fo) in enumerate(((b0_d, HID), (b1_d, HID), (b2_d, OUT))):
                bt = spool.tile([1, fo], dt.float32, name=f"b{i}sb", tag=f"b{i}")
                nc.sync.dma_start(bt[:], bd[:])
                b_sb.append(bt)

            # ---- local degree -> dinv_t (no collective) ----
            deg_sb = spool.tile([1, TS], dt.float32, name="deg_sb")
            nc.vector.tensor_copy(deg_sb[:, 0:512], deg_ps0[:])
            nc.vector.tensor_copy(deg_sb[:, 512:TS], deg_ps1[:])
            nc.sync.dma_start(deg_bounce[:], deg_sb[:])

            # sqrt(deg[t]) row for bias prefill (local slice)
            sqd_row = spool.tile([1, TS], dt.float32, name="sqd_row")
            nc.scalar.activation(sqd_row[:], deg_sb[:], AF.Sqrt)

            # dinv for own targets: [128, 8], col j covers t-block j
            dinv_t = spool.tile([P, GRP], dt.float32, name="dinv_t")
            nc.sync.dma_start(
                dinv_t[:], deg_bounce[:].rearrange("o (j p) -> p (o j)", p=P)
            )
            nc.scalar.activation(dinv_t[:], dinv_t[:], AF.Sqrt)
            nc.vector.reciprocal(dinv_t[:], dinv_t[:])
            dinv2_t = spool.tile([P, GRP], dt.float32, name="dinv2_t")
            nc.vector.tensor_mul(dinv2_t[:], dinv_t[:], dinv_t[:])

            # ---- z1 slice = dinv_t ⊙ x_slice (bf16), then AG ----
            zx_sb = zpool.tile(
                [P, GRP * IN], dt.bfloat16, name="zx_sb", tag="yout", bufs=2
            )
            for j in range(GRP):
                nc.vector.tensor_scalar_mul(
                    zx_sb[:, j * IN:(j + 1) * IN], xj[:, j, :], dinv_t[:, j:j + 1]
                )

            def ag_and_load(li, y_sb, f):
                """DMA y_sb -> zb[li], AllGather -> zall[li], load z group tiles."""
                nc.sync.dma_start(
                    zb[li][:].rearrange("(j p) f -> p j f", p=P),
                    y_sb[:].rearrange("p (j f) -> p j f", f=f),
                )
                nc.gpsimd.collective_compute(
                    "AllGather",
                    mybir.AluOpType.bypass,
                    replica_groups=rg,
                    ins=[zb[li][:].opt()],
                    outs=[zall[li][:].opt()],
                )
                zall_view = zall[li][:].rearrange(
                    "(g kk p) f -> g p kk f", g=GRP, p=P
                )
                tiles = []
                for g in range(GRP):
                    zg = zpool.tile(
                        [P, GRP * f], dt.bfloat16, name=f"z{li}g{g}",
                        tag="zg", bufs=10,
                    )
                    nc.sync.dma_start(
                        zg[:].rearrange("p (k f) -> p k f", f=f), zall_view[g]
                    )
                    tiles.append(zg)
                return tiles

            z_tiles = ag_and_load(0, zx_sb, IN)

            # ---- layers ----
            def layer(li, f_in, f_out, z_tiles_in):
                u_ps0 = pu.tile([f_in, 512], dt.float32, name=f"u{li}ps0", tag="u0")
                u_ps1 = pu.tile([f_in, 512], dt.float32, name=f"u{li}ps1", tag="u1")
                for k in range(SB):
                    zt = z_tiles_in[k // GRP][:, (k % GRP) * f_in:(k % GRP + 1) * f_in]
                    rhs = a_tiles[k // 2][:, k % 2, :]
                    nc.tensor.matmul(
                        u_ps0[:], zt, rhs[:, 0:512],
                        start=(k == 0), stop=(k == SB - 1),
                    )
                    nc.tensor.matmul(
                        u_ps1[:], zt, rhs[:, 512:TS],
                        start=(k == 0), stop=(k == SB - 1),
                    )
                u_sb = upool.tile([f_in, TS], dt.float32, name=f"u{li}sb", tag="u")
                nc.vector.tensor_copy(u_sb[:, 0:512], u_ps0[:])
                nc.vector.tensor_copy(u_sb[:, 512:TS], u_ps1[:])

                last = li == 2
                odt = dt.float32 if last else dt.bfloat16
                y_sb = zpool.tile(
                    [P, GRP * f_out], odt, name=f"y{li}sb",
                    tag="yout" if last else "zg",
                    bufs=2 if last else 10,
                )
                for j in range(GRP):
                    y_ps = py.tile([P, f_out], dt.float32, name=f"y{li}ps{j}", tag="y")
                    # PSUM prefill: sqrt(deg[t]) * b[fo]
                    nc.tensor.matmul(
                        y_ps[:],
                        sqd_row[0:1, j * P:(j + 1) * P],
                        b_sb[li][:],
                        start=True, stop=False,
                    )
                    nc.tensor.matmul(
                        y_ps[:],
                        u_sb[:, j * P:(j + 1) * P],
                        w_sb[li][:],
                        start=False, stop=True,
                    )
                    scale = (dinv_t if last else dinv2_t)[:, j:j + 1]
                    nc.scalar.activation(
                        y_sb[:, j * f_out:(j + 1) * f_out],
                        y_ps[:],
                        AF.Relu,
                        scale=scale,
                    )
                return y_sb

            for li, (f_in, f_out) in enumerate(((IN, HID), (HID, HID), (HID, OUT))):
                y_sb = layer(li, f_in, f_out, z_tiles)
                if li == 2:
                    nc.sync.dma_start(
                        out_sh[:].rearrange("(j p) f -> p j f", p=P),
                        y_sb[:].rearrange("p (j f) -> p j f", f=OUT),
                    )
                else:
                    z_tiles = ag_and_load(li + 1, y_sb, f_out)

    _split_sync_waits(nc)
    return nc


def _split_sync_waits(nc, limit=1):
    """This container's walrus rejects >1 sync wait per instruction
    ("Too many sync wait commands").  Hoist extra waits onto NoOps that
    immediately precede the instruction on the same engine — semantically
    identical since waits on one engine sequencer serialize anyway."""
    import concourse.mybir as mybir

    n_split = 0
    for f in nc.m.functions:
        for bb in f.blocks:
            out = []
            for ins in bb.instructions:
                si = ins.sync_info
                waits = list(si.on_wait) if si is not None and si.on_wait else []
                if len(waits) > limit:
                    extra, keep = waits[:-limit], waits[-limit:]
                    for i, w in enumerate(extra):
                        out.append(
                            mybir.InstNoOp(
                                name=f"{ins.name}-ws{i}",
                                engine=ins.engine,
                                bass_nofuse=True,
                                sync_info=mybir.SyncInfo(on_wait=[w], on_update=[]),
                            )
                        )
                    ins.sync_info = mybir.SyncInfo(
                        on_wait=keep, on_update=list(si.on_update)
                    )
                    n_split += 1
                out.append(ins)
            bb.instructions = out
    return n_split


def _prep_shards(bone_features, bone_adj, W0, b0, W1, b1, W2, b2):
    fp8 = ml_dtypes.float8_e4m3
    a_bin = (bone_adj != 0).astype(fp8)
    np.fill_diagonal(a_bin, fp8(1.0))
    x = np.ascontiguousarray(bone_features, dtype=np.float32)
    common = dict(
        w0=np.ascontiguousarray(W0, dtype=np.float32),
        w1=np.ascontiguousarray(W1, dtype=np.float32),
        w2=np.ascontiguousarray(W2, dtype=np.float32),
        b0=np.ascontiguousarray(b0, dtype=np.float32).reshape(1, HID),
        b1=np.ascontiguousarray(b1, dtype=np.float32).reshape(1, HID),
        b2=np.ascontiguousarray(b2, dtype=np.float32).reshape(1, OUT),
    )
    in_maps = []
    for c in range(NCORES):
        in_maps.append(
            dict(
                a_sh=np.ascontiguousarray(a_bin[:, c * TS:(c + 1) * TS]),
                x_sl=np.ascontiguousarray(x[c * TS:(c + 1) * TS]),
                **common,
            )
        )
    return in_maps


def _ensure_ntff_hook():
    """The image's antenv lacks axon_hooks, so boot() skipped registering the
    NTFF profile hook.  Recreate the module and register the hook so
    run_bass_kernel_spmd(trace=True) can profile."""
    import sys
    import types

    if "antenv.axon_hooks" in sys.modules:
        return
    mod = types.ModuleType("antenv.axon_hooks")
    hook = [None]
    mod.set_axon_ntff_profile_hook = lambda h: hook.__setitem__(0, h)
    mod.get_axon_ntff_profile_hook = lambda: hook[0]
    sys.modules["antenv.axon_hooks"] = mod
    import antenv

    antenv.axon_hooks = mod
    try:
        from trn_agent_boot.trn_boot import _ntff_profile_via_ctypes

        mod.set_axon_ntff_profile_hook(
            _ntff_profile_via_ctypes("/opt/axon/libaxon_pjrt.so")
        )
    except Exception:
        pass


def run(trace=False, **inputs):
    from concourse.bass_utils import run_bass_kernel_spmd

    if trace:
        _ensure_ntff_hook()
    if "nc" not in _CACHE:
        _CACHE["nc"] = _build()
    nc = _CACHE["nc"]
    in_maps = _prep_shards(**inputs)
    res = run_bass_kernel_spmd(
        nc, in_maps, list(range(NCORES)), trace=trace,
        trace_cores=list(range(NCORES)) if trace else None,
    )
    out = np.concatenate(
        [np.asarray(res.results[c]["out_sh"]) for c in range(NCORES)], axis=0
    )
    return out, res


def kernel(**inputs):
    out, _ = run(trace=False, **inputs)
    return out


# revision 14
# speedup vs baseline: 1.1412x; 1.0911x over previous
"""GCN (3-layer) Bass kernel for Trainium2, 8 NeuronCores.

Reference computation (B=8192, IN=64, HID=128, OUT=64):
    A = binarize(bone_adj); A[diag] = 1
    deg = A.sum(axis=0); dinv = rsqrt(deg)
    N = dinv[:,None] * A * dinv[None,:]
    x = features; for (W, b) in layers: x = relu(N.T @ (x @ W) + b)

Kernel strategy:
  - Column-shard A across 8 cores: core c owns target nodes t in
    [1024c, 1024(c+1)).  Host casts the binary A to fp8e4m3 (exact 0/1).
  - Each core keeps its whole 8MB fp8 shard resident in SBUF (loaded once).
  - deg[t] = sum_s A[s,t] for the core's own targets is a column sum of its
    own shard — computed on device via ones-vector matmuls pipelined under
    the A load.  No degree AllGather is needed: every consumer of dinv[s]
    gets it pre-applied by the owner of s (see below).
  - Activations cross cores pre-scaled: the AllGathered tensor for layer l
    is Z = dinv ⊙ Y_{l-1} (bf16), where each core scales its own slice with
    its locally-computed dinv.  Layer l computes:
      U_t = Z.T @ A_shard        (PE: Z s-block stationary bf16, A moving fp8)
      Y_pre[t, fo] = (U @ W)[t, fo] + sqrt(deg[t]) * b[fo]  (small f32 matmuls;
        the sqrt(deg) prefactor cancels the later dinv_t scale on the bias)
      AG input     = relu(dinv_t^2 * Y_pre)   (= dinv ⊙ relu(dinv ⊙ (UW) + b))
      final output = relu(dinv_t   * Y_pre)
  - A short burst of dummy matmuls at kernel start warms the PE clock gate
    (HAM) so the degree pass runs at 2.4 GHz.
"""

import numpy as np
import ml_dtypes

B, IN, HID, OUT = 8192, 64, 128, 64
NCORES = 8
TS = B // NCORES  # 1024 targets per core
P = 128
SB = B // P  # 64 source blocks
NPAIR = SB // 2  # A held as 32 pair-tiles [128, 2, 1024]
GRP = 8  # z group tiles: 8 groups of 8 s-blocks

_CACHE = {}


def _build():
    import concourse.bass as bass
    import concourse.mybir as mybir
    import concourse.tile as tile

    dt = mybir.dt
    AF = mybir.ActivationFunctionType

    nc = bass.Bass(num_devices=NCORES)

    # ---- I/O ----
    a_sh = nc.dram_tensor("a_sh", [B, TS], dt.float8e4, kind="ExternalInput")
    x_sl = nc.dram_tensor("x_sl", [TS, IN], dt.float32, kind="ExternalInput")
    w0_d = nc.dram_tensor("w0", [IN, HID], dt.bfloat16, kind="ExternalInput")
    w1_d = nc.dram_tensor("w1", [HID, HID], dt.bfloat16, kind="ExternalInput")
    w2_d = nc.dram_tensor("w2", [HID, OUT], dt.bfloat16, kind="ExternalInput")
    b0_d = nc.dram_tensor("b0", [1, HID], dt.bfloat16, kind="ExternalInput")
    b1_d = nc.dram_tensor("b1", [1, HID], dt.bfloat16, kind="ExternalInput")
    b2_d = nc.dram_tensor("b2", [1, OUT], dt.bfloat16, kind="ExternalInput")
    out_sh = nc.dram_tensor("out_sh", [TS, OUT], dt.float32, kind="ExternalOutput")

    rg = [list(range(NCORES))]

    with tile.TileContext(nc) as tc:
        with (
            tc.tile_pool(name="dram", bufs=1, space="DRAM") as dram,
            tc.tile_pool(name="apool", bufs=NPAIR) as apool,
            tc.tile_pool(name="zpool", bufs=1) as zpool,
            tc.tile_pool(name="spool", bufs=1) as spool,
            tc.tile_pool(name="upool", bufs=2) as upool,
            tc.tile_pool(name="pdeg", bufs=1, space="PSUM") as pdeg,
            tc.tile_pool(name="pu", bufs=1, space="PSUM") as pu,
            tc.tile_pool(name="py", bufs=2, space="PSUM") as py,
        ):
            # ---- collective bounce buffers (DRAM) ----
            fdims = (IN, HID, HID)
            zb = [
                dram.tile([TS, fdims[i]], dt.bfloat16, name=f"zb{i}", tag=f"zb{i}")
                for i in range(3)
            ]
            zall = [
                dram.tile(
                    [B, fdims[i]], dt.bfloat16, addr_space="Shared",
                    name=f"zall{i}", tag=f"zall{i}",
                )
                for i in range(3)
            ]

            # ---- constants + PE warmup (HAM: ~4us of matmuls -> 2.4 GHz) ----
            ones_bf = spool.tile([P, 1], dt.bfloat16, name="ones_bf")
            nc.vector.memset(ones_bf[:], 1.0)
            junk = spool.tile([P, 512], dt.bfloat16, name="junk")
            nc.vector.memset(junk[:], 0.0)
            import os as _os
            if not _os.environ.get("K_NO_WARMUP"):
                wu_ps = pdeg.tile([1, 512], dt.float32, name="wu_ps", tag="wu")
                for _ in range(20):
                    nc.tensor.matmul(
                        wu_ps[:], ones_bf[:], junk[:], start=True, stop=True
                    )

            # ---- phase A: load A shard (SBUF-resident, fp8) + degree pass ----
            a_tiles = []
            a_view = a_sh[:].rearrange("(k h p) t -> k p h t", h=2, p=P)
            deg_ps0 = pdeg.tile([1, 512], dt.float32, name="deg_ps0", tag="deg0")
            deg_ps1 = pdeg.tile([1, 512], dt.float32, name="deg_ps1", tag="deg1")
            for k in range(NPAIR):
                at = apool.tile([P, 2, TS], dt.float8e4, name=f"a{k}", tag="a")
                nc.sync.dma_start(at[:], a_view[k])
                a_tiles.append(at)
            for k in range(NPAIR):
                for h in range(2):
                    nc.tensor.matmul(
                        deg_ps0[:], ones_bf[:], a_tiles[k][:, h, 0:512],
                        start=(k == 0 and h == 0), stop=(k == NPAIR - 1 and h == 1),
                    )
                    nc.tensor.matmul(
                        deg_ps1[:], ones_bf[:], a_tiles[k][:, h, 512:TS],
                        start=(k == 0 and h == 0), stop=(k == NPAIR - 1 and h == 1),
                    )

            # prefetch x / W / b while A loads
            xj = spool.tile([P, GRP, IN], dt.float32, name="xj")
            nc.sync.dma_start(
                xj[:], x_sl[:].rearrange("(j p) f -> p j f", p=P)
            )
            w_sb = []
            for i, (wd, fi, fo) in enumerate(
                ((w0_d, IN, HID), (w1_d, HID, HID), (w2_d, HID, OUT))
            ):
                wt = spool.tile([fi, fo], dt.bfloat16, name=f"w{i}sb", tag=f"w{i}")
                nc.sync.dma_start(wt[:], wd[:])
                w_sb.append(wt)
            b_sb = []
            for i, (bd, fo) in enumerate(((b0_d, HID), (b1_d, HID), (b2_d, OUT))):
                bt = spool.tile([1, fo], dt.bfloat16, name=f"b{i}sb", tag=f"b{i}")
                nc.sync.dma_start(bt[:], bd[:])
                b_sb.append(bt)

            # ---- local degree -> dinv_t (no collective) ----
            deg_sb = spool.tile([1, TS], dt.float32, name="deg_sb")
            nc.vector.tensor_copy(deg_sb[:, 0:512], deg_ps0[:])
            nc.vector.tensor_copy(deg_sb[:, 512:TS], deg_ps1[:])

            # sqrt(deg[t]) row for bias prefill (local slice)
            sqd_row = spool.tile([1, TS], dt.bfloat16, name="sqd_row")
            nc.scalar.activation(sqd_row[:], deg_sb[:], AF.Sqrt)

            # dinv for own targets: [128, 8] via K=1 transpose matmuls
            one_f32 = spool.tile([1, 1], dt.float32, name="one_f32")
            nc.vector.memset(one_f32[:], 1.0)
            dt_ps = py.tile([P, GRP], dt.float32, name="dt_ps", tag="dtp", bufs=1)
            for j in range(GRP):
                nc.tensor.matmul(
                    dt_ps[:, j:j + 1],
                    deg_sb[0:1, j * P:(j + 1) * P],
                    one_f32[:],
                    start=True, stop=True,
                )
            dinv_t = spool.tile([P, GRP], dt.float32, name="dinv_t")
            nc.scalar.activation(dinv_t[:], dt_ps[:], AF.Sqrt)
            nc.vector.reciprocal(dinv_t[:], dinv_t[:])
            dinv2_t = spool.tile([P, GRP], dt.float32, name="dinv2_t")
            nc.vector.tensor_mul(dinv2_t[:], dinv_t[:], dinv_t[:])

            # ---- z1 slice = dinv_t ⊙ x_slice (bf16), then AG ----
            zx_sb = zpool.tile(
                [P, GRP * IN], dt.bfloat16, name="zx_sb", tag="yout", bufs=2
            )
            for j in range(GRP):
                nc.vector.tensor_scalar_mul(
                    zx_sb[:, j * IN:(j + 1) * IN], xj[:, j, :], dinv_t[:, j:j + 1]
                )

            def ag_and_load(li, y_sb, f):
                """DMA y_sb -> zb[li], AllGather -> zall[li], load z group tiles."""
                nc.sync.dma_start(
                    zb[li][:].rearrange("(j p) f -> p j f", p=P),
                    y_sb[:].rearrange("p (j f) -> p j f", f=f),
                )
                nc.gpsimd.collective_compute(
                    "AllGather",
                    mybir.AluOpType.bypass,
                    replica_groups=rg,
                    ins=[zb[li][:].opt()],
                    outs=[zall[li][:].opt()],
                )
                zall_view = zall[li][:].rearrange(
                    "(g kk p) f -> g p kk f", g=GRP, p=P
                )
                tiles = []
                for g in range(GRP):
                    zg = zpool.tile(
                        [P, GRP * f], dt.bfloat16, name=f"z{li}g{g}",
                        tag="zg", bufs=10,
                    )
                    nc.sync.dma_start(
                        zg[:].rearrange("p (k f) -> p k f", f=f), zall_view[g]
                    )
                    tiles.append(zg)
                return tiles

            z_tiles = ag_and_load(0, zx_sb, IN)

            # ---- layers ----
            def layer(li, f_in, f_out, z_tiles_in):
                u_ps0 = pu.tile([f_in, 512], dt.float32, name=f"u{li}ps0", tag="u0")
                u_ps1 = pu.tile([f_in, 512], dt.float32, name=f"u{li}ps1", tag="u1")
                for k in range(SB):
                    zt = z_tiles_in[k // GRP][:, (k % GRP) * f_in:(k % GRP + 1) * f_in]
                    rhs = a_tiles[k // 2][:, k % 2, :]
                    nc.tensor.matmul(
                        u_ps0[:], zt, rhs[:, 0:512],
                        start=(k == 0), stop=(k == SB - 1),
                    )
                    nc.tensor.matmul(
                        u_ps1[:], zt, rhs[:, 512:TS],
                        start=(k == 0), stop=(k == SB - 1),
                    )
                u_sb = upool.tile([f_in, TS], dt.bfloat16, name=f"u{li}sb", tag="u")
                nc.vector.tensor_copy(u_sb[:, 0:512], u_ps0[:])
                nc.vector.tensor_copy(u_sb[:, 512:TS], u_ps1[:])

                last = li == 2
                odt = dt.float32 if last else dt.bfloat16
                y_sb = zpool.tile(
                    [P, GRP * f_out], odt, name=f"y{li}sb",
                    tag="yout" if last else "zg",
                    bufs=2 if last else 10,
                )
                for j in range(GRP):
                    y_ps = py.tile([P, f_out], dt.float32, name=f"y{li}ps{j}", tag="y")
                    # PSUM prefill: sqrt(deg[t]) * b[fo]
                    nc.tensor.matmul(
                        y_ps[:],
                        sqd_row[0:1, j * P:(j + 1) * P],
                        b_sb[li][:],
                        start=True, stop=False,
                    )
                    nc.tensor.matmul(
                        y_ps[:],
                        u_sb[:, j * P:(j + 1) * P],
                        w_sb[li][:],
                        start=False, stop=True,
                    )
                    scale = (dinv_t if last else dinv2_t)[:, j:j + 1]
                    nc.scalar.activation(
                        y_sb[:, j * f_out:(j + 1) * f_out],
                        y_ps[:],
                        AF.Relu,
                        scale=scale,
                    )
                return y_sb

            for li, (f_in, f_out) in enumerate(((IN, HID), (HID, HID), (HID, OUT))):
                y_sb = layer(li, f_in, f_out, z_tiles)
                if li == 2:
                    nc.sync.dma_start(
                        out_sh[:].rearrange("(j p) f -> p j f", p=P),
                        y_sb[:].rearrange("p (j f) -> p j f", f=OUT),
                    )
                else:
                    z_tiles = ag_and_load(li + 1, y_sb, f_out)

    _split_sync_waits(nc)
    return nc


def _split_sync_waits(nc, limit=1):
    """This container's walrus rejects >1 sync wait per instruction
    ("Too many sync wait commands").  Hoist extra waits onto NoOps that
    immediately precede the instruction on the same engine — semantically
    identical since waits on one engine sequencer serialize anyway."""
    import concourse.mybir as mybir

    n_split = 0
    for f in nc.m.functions:
        for bb in f.blocks:
            out = []
            for ins in bb.instructions:
                si = ins.sync_info
                waits = list(si.on_wait) if si is not None and si.on_wait else []
                if len(waits) > limit:
                    extra, keep = waits[:-limit], waits[-limit:]
                    for i, w in enumerate(extra):
                        out.append(
                            mybir.InstNoOp(
                                name=f"{ins.name}-ws{i}",
                                engine=ins.engine,
                                bass_nofuse=True,
                                sync_info=mybir.SyncInfo(on_wait=[w], on_update=[]),
                            )
                        )
                    ins.sync_info = mybir.SyncInfo(
                        on_wait=keep, on_update=list(si.on_update)
                    )
                    n_split += 1
                out.append(ins)
            bb.instructions = out
    return n_split


def _prep_shards(bone_features, bone_adj, W0, b0, W1, b1, W2, b2):
    fp8 = ml_dtypes.float8_e4m3
    a_bin = (bone_adj != 0).astype(fp8)
    np.fill_diagonal(a_bin, fp8(1.0))
    x = np.ascontiguousarray(bone_features, dtype=np.float32)
    bf16 = ml_dtypes.bfloat16
    common = dict(
        w0=np.ascontiguousarray(W0, dtype=bf16),
        w1=np.ascontiguousarray(W1, dtype=bf16),
        w2=np.ascontiguousarray(W2, dtype=bf16),
        b0=np.ascontiguousarray(b0, dtype=bf16).reshape(1, HID),
        b1=np.ascontiguousarray(b1, dtype=bf16).reshape(1, HID),
        b2=np.ascontiguousarray(b2, dtype=bf16).reshape(1, OUT),
    )
    in_maps = []
    for c in range(NCORES):
        in_maps.append(
            dict(
                a_sh=np.ascontiguousarray(a_bin[:, c * TS:(c + 1) * TS]),
                x_sl=np.ascontiguousarray(x[c * TS:(c + 1) * TS]),
                **common,
            )
        )
    return in_maps


def _ensure_ntff_hook():
    """The image's antenv lacks axon_hooks, so boot() skipped registering the
    NTFF profile hook.  Recreate the module and register the hook so
    run_bass_kernel_spmd(trace=True) can profile."""
    import sys
    import types

    if "antenv.axon_hooks" in sys.modules:
        return
    mod = types.ModuleType("antenv.axon_hooks")
    hook = [None]
    mod.set_axon_ntff_profile_hook = lambda h: hook.__setitem__(0, h)
    mod.get_axon_ntff_profile_hook = lambda: hook[0]
    sys.modules["antenv.axon_hooks"] = mod
    import antenv

    antenv.axon_hooks = mod
    try:
        from trn_agent_boot.trn_boot import _ntff_profile_via_ctypes

        mod.set_axon_ntff_profile_hook(
            _ntff_profile_via_ctypes("/opt/axon/libaxon_pjrt.so")
        )
    except Exception:
        pass


def run(trace=False, **inputs):
    from concourse.bass_utils import run_bass_kernel_spmd

    if trace:
        _ensure_ntff_hook()
    if "nc" not in _CACHE:
        _CACHE["nc"] = _build()
    nc = _CACHE["nc"]
    in_maps = _prep_shards(**inputs)
    res = run_bass_kernel_spmd(
        nc, in_maps, list(range(NCORES)), trace=trace,
        trace_cores=list(range(NCORES)) if trace else None,
    )
    out = np.concatenate(
        [np.asarray(res.results[c]["out_sh"]) for c in range(NCORES)], axis=0
    )
    return out, res


def kernel(**inputs):
    out, _ = run(trace=False, **inputs)
    return out
